# revision 1
# baseline (speedup 1.0000x reference)
"""Per-sample Gaussian blur (inverse-heat-dissipation style) as banded matmuls on TRN2.

Formulation: for each sample b, the separable blur with reflect padding is
    out[b, c] = M_b @ x[b, c] @ M_b^T
where M_b [512, 512] is the 1-D blur operator with the reflect boundary folded
in (row i: the 161-tap Gaussian centered at i, reflected at the edges).

On the PE array (out = lhsT.T @ rhs, lhsT stationary, rhs moving) both passes
run transpose-free with the SAME rhs matrix M_T = M_b^T ([input idx, output idx]):
    pass 1: A_T = lhsT(X).T @ M_T      -> A_T[w, h]   (blur along h, transposed)
    pass 2: Z   = lhsT(A_T).T @ M_T    -> Z[h, w_out] (blur along w)

M_T is banded (|i-j| <= r_eff + reflect corners), so each K-block of the
contraction only touches a narrow column band of the output: matmuls stream
only that band, accumulating different column ranges into one PSUM bank
(start=True clears the whole bank's has_written bits; later matmuls overwrite
unwritten columns, accumulate written ones).

Sharding: pure data parallel over batch, 8 samples/core. Samples are sorted by
sigma and dealt so slot s holds 8 similar sigmas across cores; the single SPMD
program uses per-slot bands sized to the slot max sigma.
"""

import numpy as np
import ml_dtypes

import concourse.bass as bass
import concourse.bacc as bacc
import concourse.mybir as mybir
import concourse.tile as tile
from concourse.bass_utils import run_bass_kernel_spmd

B, C, H, W = 64, 3, 512, 512
NCORES = 8
SPB = B // NCORES          # samples per core
P = 128
NT = H // P                # 4 row/col blocks of 128
RADIUS = 80
KSIZE = 2 * RADIUS + 1
TAP_THRESHOLD = 1e-6       # taps below this (relative to unit row sum) are dropped

MM_DT = "bf16"             # "bf16" | "f32r" | "f32"
Y_DT = "bf16"              # "bf16" | "f32": on-wire output dtype (host upcasts)

_DT = {
    "bf16": mybir.dt.bfloat16,
    "f32r": mybir.dt.float32,
    "f32": mybir.dt.float32,
}
_NPDT = {
    "bf16": ml_dtypes.bfloat16,
    "f32r": np.float32,
    "f32": np.float32,
}


def _gauss_k1d(blur_sigmas: np.ndarray, fwd_steps: np.ndarray) -> np.ndarray:
    sig = blur_sigmas.astype(np.float64)[fwd_steps] + 1e-6
    half = (KSIZE - 1) / 2.0
    t = np.linspace(-half, half, KSIZE)
    pdf = np.exp(-0.5 * (t[None, :] / sig[:, None]) ** 2)
    return pdf / pdf.sum(axis=1, keepdims=True)  # [B, K] float64


def _blur_matrices(k1d: np.ndarray) -> np.ndarray:
    """M[b] (float64): out = M @ x along one axis, reflect padding folded in."""
    nb = k1d.shape[0]
    i = np.arange(H)[:, None]
    j = i - RADIUS + np.arange(KSIZE)[None, :]
    jr = np.abs(j)                                   # reflect at 0
    jr = np.where(jr > H - 1, 2 * (H - 1) - jr, jr)  # reflect at H-1
    ii = np.broadcast_to(i, jr.shape)
    M = np.zeros((nb, H, H), np.float64)
    for b in range(nb):
        np.add.at(M[b], (ii, jr), np.broadcast_to(k1d[b][None, :], jr.shape))
    return M


def _slot_bands(M_slot: np.ndarray, min_w: int) -> list[tuple[int, int]]:
    """Per K-block output-column band [lo, hi) covering all samples in a slot.

    Band ki = rows of M where columns [128ki, 128ki+128) have any entry above
    threshold. Always contains [128ki, 128ki+128) (the diagonal), so adjacent
    bands overlap and their union covers [0, H).
    """
    bands = []
    for ki in range(NT):
        blk = np.abs(M_slot[:, :, ki * P : (ki + 1) * P])
        rows = np.nonzero(blk.max(axis=(0, 2)) > TAP_THRESHOLD)[0]
        lo = min(int(rows.min()), ki * P)
        hi = max(int(rows.max()) + 1, ki * P + P)
        if hi - lo < min_w:  # float32r needs moving dim >= 256 for full rate
            lo = max(0, lo - (min_w - (hi - lo)))
            hi = min(H, max(hi, lo + min_w))
            lo = max(0, min(lo, hi - min_w))
        lo &= ~1
        hi = min(H, (hi + 1) & ~1)
        bands.append((lo, hi))
    return bands


def _build(bands: list[list[tuple[int, int]]]) -> bass.Bass:
    """bands[s][ki] = (lo, hi) output-column band of M_T K-block ki for slot s.

    DRAM layouts are the exact SBUF tile layouts (host repacks):
      x  [SPB, C, P, NT*W]  : row p holds the NT K-block rows concatenated
      mt [sum_s P*TW_s]     : per slot, [P, TW_s] of banded M_T columns
      y  [SPB, C, P, NT*W]  : same blocked layout as x, fp32
    """
    nc = bacc.Bacc(None, target_bir_lowering=False)
    mmdt = _DT[MM_DT]
    f32 = mybir.dt.float32
    tws = [sum(hi - lo for lo, hi in bands[s]) for s in range(SPB)]
    x_d = nc.declare_dram_parameter("x", [SPB, C, P, NT * W], mmdt, isOutput=False)
    mt_d = nc.declare_dram_parameter("mt", [P * sum(tws)], mmdt, isOutput=False)
    ydt = mybir.dt.bfloat16 if Y_DT == "bf16" else f32
    y_d = nc.declare_dram_parameter("y", [SPB, C, P, NT * W], ydt, isOutput=True)

    def mm_ap(ap):
        return ap.bitcast(mybir.dt.float32r) if MM_DT == "f32r" else ap

    with tile.TileContext(nc) as tc:
        with (
            tc.tile_pool(name="mtp", bufs=2) as mtp,
            tc.tile_pool(name="xp", bufs=6) as xp,
            tc.tile_pool(name="atp", bufs=3) as atp,
            tc.tile_pool(name="otp", bufs=6) as otp,
            tc.tile_pool(name="pp", bufs=8, space="PSUM") as pp,
        ):
            mt_ofs = 0
            for s in range(SPB):
                offs = [0]
                for lo, hi in bands[s]:
                    offs.append(offs[-1] + (hi - lo))
                mt_t = mtp.tile([P, tws[s]], mmdt, tag="mt", name=f"mt{s}")
                nc.sync.dma_start(
                    out=mt_t[:],
                    in_=mt_d[mt_ofs : mt_ofs + P * tws[s]].rearrange(
                        "(p t) -> p t", p=P
                    ),
                )
                mt_ofs += P * tws[s]
                for c in range(C):
                    x_t = xp.tile([P, NT * W], mmdt, tag="x", name=f"x{s}_{c}")
                    nc.sync.dma_start(out=x_t[:], in_=x_d[s, c])
                    # pass 1: A_T[w, h] = X^T @ M^T, one PSUM bank per w-block
                    a_ts = [
                        atp.tile([P, H], mmdt, tag=f"a{mi}", name=f"a{s}_{c}_{mi}")
                        for mi in range(NT)
                    ]
                    for mi in range(NT):
                        ps = pp.tile([P, H], f32, tag="ps", name=f"ps{s}_{c}_{mi}")
                        for ki in range(NT):
                            lo, hi = bands[s][ki]
                            nc.tensor.matmul(
                                ps[:, lo:hi],
                                lhsT=mm_ap(
                                    x_t[:, ki * W + mi * P : ki * W + (mi + 1) * P]
                                ),
                                rhs=mm_ap(mt_t[:, offs[ki] : offs[ki + 1]]),
                                start=(ki == 0),
                                stop=(ki == NT - 1),
                            )
                        nc.scalar.copy(out=a_ts[mi][:], in_=ps[:])
                    # pass 2: Z[h, w_out] = A @ M^T
                    o_t = otp.tile([P, NT * W], ydt, tag="o", name=f"o{s}_{c}")
                    for mi in range(NT):
                        ps = pp.tile([P, H], f32, tag="ps", name=f"ps{s}_{c}_{mi}")
                        for ki in range(NT):
                            lo, hi = bands[s][ki]
                            nc.tensor.matmul(
                                ps[:, lo:hi],
                                lhsT=mm_ap(a_ts[ki][:, mi * P : (mi + 1) * P]),
                                rhs=mm_ap(mt_t[:, offs[ki] : offs[ki + 1]]),
                                start=(ki == 0),
                                stop=(ki == NT - 1),
                            )
                        nc.vector.tensor_copy(
                            out=o_t[:, mi * W : (mi + 1) * W], in_=ps[:]
                        )
                    nc.gpsimd.dma_start(out=y_d[s, c], in_=o_t[:])

    nc.finalize()
    return nc


def _prepare(x, blur_sigmas, fwd_steps):
    x = np.asarray(x, dtype=np.float32)
    blur_sigmas = np.asarray(blur_sigmas, dtype=np.float32)
    fwd_steps = np.asarray(fwd_steps, dtype=np.int32)

    k1d = _gauss_k1d(blur_sigmas, fwd_steps)
    M = _blur_matrices(k1d)
    sig = blur_sigmas.astype(np.float64)[fwd_steps]
    # slot s on core m handles global sample asn[s, m]; sorting by sigma keeps
    # per-slot bands tight across cores
    asn = np.argsort(sig, kind="stable").reshape(SPB, NCORES)

    min_w = 2 if MM_DT == "bf16" else 256
    bands = [_slot_bands(M[asn[s]], min_w) for s in range(SPB)]

    npdt = _NPDT[MM_DT]
    in_maps = []
    for m in range(NCORES):
        gs = asn[:, m]
        # x in SBUF layout: [SPB, C, P, NT*W], K-block rows concatenated
        xs = (
            x[gs]
            .reshape(SPB, C, NT, P, W)
            .transpose(0, 1, 3, 2, 4)
            .reshape(SPB, C, P, NT * W)
            .astype(npdt)
        )
        # mt: per slot a [P, TW_s] block of banded M_T columns, flattened
        parts = []
        for s in range(SPB):
            Ms = M[asn[s, m]]
            blk = [
                Ms[lo:hi, ki * P : (ki + 1) * P].T
                for ki, (lo, hi) in enumerate(bands[s])
            ]
            parts.append(np.concatenate(blk, axis=1).astype(npdt).ravel())
        in_maps.append({"x": xs, "mt": np.concatenate(parts)})
    return asn, bands, in_maps


def kernel(x, blur_sigmas, fwd_steps, _trace=False, _trace_cores=None):
    asn, bands, in_maps = _prepare(x, blur_sigmas, fwd_steps)
    nc = _build(bands)
    br = run_bass_kernel_spmd(
        nc,
        in_maps,
        list(range(NCORES)),
        trace=_trace,
        trace_cores=_trace_cores,
    )
    y = np.empty((B, C, H, W), np.float32)
    for m in range(NCORES):
        yc = br.results[m]["y"].astype(np.float32).reshape(SPB, C, P, NT, W)
        y[asn[:, m]] = yc.transpose(0, 1, 3, 2, 4).reshape(SPB, C, H, W)
    if _trace:
        kernel.last_results = br  # stash for the harness to read exec_time_ns
    return y

